# revision 1
# baseline (speedup 1.0000x reference)
"""Trainium2 Bass kernel for a 3-layer GCN (GCNConv x3 + global mean pool + linear head).

Strategy (8 NeuronCores, SPMD single program):
- Nodes sharded across 8 cores (6250 each, contiguous original ids). Within
  a core, nodes are packed into bins of M=14 nodes whose in-edge slots
  (in-edges + self loop), split by source half (cores 0-3 vs 4-7), fit in
  128 slots per half. All feature math is fp32 on device.
- Normalization folded: u = (h @ W) * dinv[src] (applied on device); the
  per-bin selection matrices U carry dinv[dst], so the TensorEngine
  segment-sum (G^T @ U accumulated over the lo/hi halves in PSUM) yields
  dinv[dst] * sum(u[src]) in feature-major layout directly.
- Per layer: u^T = W^T @ h^T (PE), PE-transpose to node-major, scale by
  dinv (DVE), DMA to HBM, AllGather, then dma_gather (SWDGE custom op,
  int16 indices -> the u table is addressed as two 26624-row halves) feeds
  the PE segment-sum; ScalarE applies bias+ReLU writing h^T in place.
- Global mean pool via a count-folded one-hot pooling matmul, small head
  matmul, AllReduce of the [64, 8] partials.
"""
import numpy as np
import sys

if "/opt/trn_rl_repo" not in sys.path:
    sys.path.insert(0, "/opt/trn_rl_repo")

import concourse.bass as bass
import concourse.bacc as bacc
import concourse.mybir as mybir
import concourse.tile as tile
from concourse.masks import make_identity
from concourse.bass_utils import run_bass_kernel_spmd

N, E, DIN, H, NGRAPH, OUT = 50000, 800000, 128, 128, 64, 8
NCORES = 8
SHARD = N // NCORES
M_COLS = 14              # nodes per bin (14*36 = 504 psum cols + 8 filler)
CHUNK_SLOTS = 128
ST_BINS = 36             # bins per 512-column PSUM supertile
PSUM_COLS = 512
GB_BINS = 8              # bins per dma_gather instruction (1024 idxs, ring cap)
NQ = 4                   # SWDGE queues round-robined for desc-gen overlap

F32 = mybir.dt.float32
I16 = mybir.dt.int16


# ----------------------------------------------------------------- host prep
def _preprocess(edge_index, batch):
    src = np.asarray(edge_index[0], dtype=np.int64)
    dst = np.asarray(edge_index[1], dtype=np.int64)
    batch = np.asarray(batch, dtype=np.int64)

    dst_counts = np.bincount(dst, minlength=N)
    deg = dst_counts.astype(np.float64) + 1.0
    dinv = (1.0 / np.sqrt(deg)).astype(np.float32)
    cnt = np.bincount(batch, minlength=NGRAPH).astype(np.float64)
    inv_cnt = (1.0 / np.maximum(cnt, 1.0)).astype(np.float32)

    order = np.argsort(dst, kind="stable")
    src_sorted = src[order]
    dst_starts = np.zeros(N + 1, dtype=np.int64)
    np.cumsum(dst_counts, out=dst_starts[1:])

    # per-node lo/hi in-slot counts (self loop counts in the node's own half)
    lo_cnt = np.zeros(N, np.int64)
    hi_cnt = np.zeros(N, np.int64)
    src_is_lo = src < (N // 2)
    np.add.at(lo_cnt, dst[src_is_lo], 1)
    np.add.at(hi_cnt, dst[~src_is_lo], 1)
    self_lo = np.arange(N) < (N // 2)
    lo_cnt += self_lo
    hi_cnt += ~self_lo

    per_core_bins = []
    for c in range(NCORES):
        lo = c * SHARD
        nodes = np.arange(lo, lo + SHARD)
        d = (dst_counts[lo:lo + SHARD] + 1).astype(np.int64)
        order_d = np.argsort(-d, kind="stable")
        B = -(-SHARD // M_COLS)
        while True:
            bins_nodes = [[] for _ in range(B)]
            for r in range(M_COLS):
                idxs = order_d[r * B:(r + 1) * B]
                for i, oi in enumerate(idxs):
                    bi = (B - 1 - i) if (r % 2 == 1) else i
                    bins_nodes[bi].append(nodes[oi])
            load_lo = np.array([sum(lo_cnt[v] for v in bn) for bn in bins_nodes])
            load_hi = np.array([sum(hi_cnt[v] for v in bn) for bn in bins_nodes])
            for _ in range(40000):
                worst = np.maximum(load_lo, load_hi)
                hi_b = int(np.argmax(worst))
                if worst[hi_b] <= CHUNK_SLOTS:
                    break
                # which half overflows decides the swap metric
                use_lo = load_lo[hi_b] >= load_hi[hi_b]
                cc = lo_cnt if use_lo else hi_cnt
                lo_b = int(np.argmin(np.maximum(load_lo, load_hi)))
                if lo_b == hi_b:
                    break
                vh = max(bins_nodes[hi_b], key=lambda v: cc[v])
                vl = min(bins_nodes[lo_b], key=lambda v: cc[v])
                if cc[vh] - cc[vl] <= 0:
                    break
                bins_nodes[hi_b].remove(vh); bins_nodes[hi_b].append(vl)
                bins_nodes[lo_b].remove(vl); bins_nodes[lo_b].append(vh)
                for arr, cnts in ((load_lo, lo_cnt), (load_hi, hi_cnt)):
                    arr[hi_b] += cnts[vl] - cnts[vh]
                    arr[lo_b] += cnts[vh] - cnts[vl]
            if max(np.maximum(load_lo, load_hi).max(), 0) <= CHUNK_SLOTS:
                break
            B = B + max(1, B // 100)
        per_core_bins.append(bins_nodes)

    nbins_max = max(len(b) for b in per_core_bins)
    NBINS = -(-nbins_max // ST_BINS) * ST_BINS
    NST = NBINS // ST_BINS
    P_pos = NST * PSUM_COLS
    TP = P_pos // 128
    HALF_ROW = (NCORES // 2) * P_pos
    NGI = -(-NBINS // GB_BINS)          # gather instrs per stream per layer

    pos_of_node = np.full(N, -1, dtype=np.int64)
    core_of_node = np.full(N, -1, dtype=np.int64)
    for c in range(NCORES):
        for j, bn in enumerate(per_core_bins[c]):
            base = (j // ST_BINS) * PSUM_COLS + (j % ST_BINS) * M_COLS
            for t, v in enumerate(bn):
                pos_of_node[v] = base + t
                core_of_node[v] = c
    assert (pos_of_node >= 0).all()
    grow_of_node = core_of_node * P_pos + pos_of_node

    per_core = []
    for c in range(NCORES):
        bins_nodes = per_core_bins[c]
        # flat slot streams (value = table-relative row), then wrap per instr
        flatA = np.zeros(NGI * GB_BINS * 128, np.int64)
        flatB = np.zeros(NGI * GB_BINS * 128, np.int64)
        ucols = np.zeros((CHUNK_SLOTS, 2 * NBINS * M_COLS + 8), dtype=np.float32)
        UC_B = NBINS * M_COLS
        for j, bn in enumerate(bins_nodes):
            sA = sB = 0
            for t, v in enumerate(bn):
                st0, en0 = dst_starts[v], dst_starts[v + 1]
                srcs = np.concatenate([src_sorted[st0:en0], [v]])
                g = grow_of_node[srcs]
                glo = g[g < HALF_ROW]
                ghi = g[g >= HALF_ROW] - HALF_ROW
                flatA[j * 128 + sA: j * 128 + sA + len(glo)] = glo
                ucols[sA:sA + len(glo), j * M_COLS + t] = dinv[v]
                sA += len(glo)
                flatB[j * 128 + sB: j * 128 + sB + len(ghi)] = ghi
                ucols[sB:sB + len(ghi), UC_B + j * M_COLS + t] = dinv[v]
                sB += len(ghi)
            assert sA <= 128 and sB <= 128

        def wrap_stream(flat):
            out = np.zeros((128, NGI * 64), np.int16)
            for b in range(NGI):
                v = flat[b * 1024:(b + 1) * 1024]
                w = v.reshape(64, 16).T.astype(np.int16)
                out[:, b * 64:(b + 1) * 64] = np.tile(w, (8, 1))
            return out

        gidxA = wrap_stream(flatA)
        gidxB = wrap_stream(flatB)

        dinv_col = np.zeros((128, TP), dtype=np.float32)
        pmat = np.zeros((128, TP * NGRAPH), dtype=np.float32)
        node_order = np.zeros(P_pos, np.int64)
        has_node = np.zeros(P_pos, bool)
        mask = core_of_node == c
        vnodes = np.nonzero(mask)[0]
        vpos = pos_of_node[vnodes]
        pp, tt = vpos % 128, vpos // 128
        dinv_col[pp, tt] = dinv[vnodes]
        pmat[pp, tt * NGRAPH + batch[vnodes]] = inv_cnt[batch[vnodes]]
        node_order[vpos] = vnodes
        has_node[vpos] = True
        per_core.append(dict(gidxA=gidxA, gidxB=gidxB, ucols=ucols,
                             dinv_col=dinv_col, pmat=pmat,
                             node_order=node_order, has_node=has_node))

    meta = dict(NBINS=NBINS, NST=NST, P_pos=P_pos, TP=TP, NGI=NGI,
                HALF_ROW=HALF_ROW)
    return meta, per_core


# -------------------------------------------------------------- device build
def _build(meta):
    NBINS, NST, P_pos, TP = meta["NBINS"], meta["NST"], meta["P_pos"], meta["TP"]
    NGI, HALF_ROW = meta["NGI"], meta["HALF_ROW"]
    UC_B = NBINS * M_COLS
    ZOFF = 2 * NBINS * M_COLS

    nc = bacc.Bacc("TRN2", target_bir_lowering=False, debug=False,
                   num_devices=NCORES, num_swdge_queues=NQ)

    xg_d = nc.dram_tensor("xg", [P_pos, 128], F32, kind="ExternalInput")
    wt_d = nc.dram_tensor("wt", [128, 3 * H], F32, kind="ExternalInput")
    wh_d = nc.dram_tensor("wh", [128, OUT], F32, kind="ExternalInput")
    bvec_d = nc.dram_tensor("bvec", [128, 3], F32, kind="ExternalInput")
    bhb_d = nc.dram_tensor("bhb", [NGRAPH, OUT], F32, kind="ExternalInput")
    giA_d = nc.dram_tensor("gidxA", [128, NGI * 64], I16, kind="ExternalInput")
    giB_d = nc.dram_tensor("gidxB", [128, NGI * 64], I16, kind="ExternalInput")
    ucols_d = nc.dram_tensor("ucols", [128, ZOFF + 8], F32, kind="ExternalInput")
    dinv_d = nc.dram_tensor("dinv", [128, TP], F32, kind="ExternalInput")
    pmat_d = nc.dram_tensor("pmat", [128, TP * NGRAPH], F32, kind="ExternalInput")
    out_d = nc.dram_tensor("out", [NGRAPH, OUT], F32, kind="ExternalOutput")

    u_shard = nc.dram_tensor("u_shard", [P_pos, 128], F32)
    u_full = nc.dram_tensor("u_full", [NCORES * P_pos, 128], F32,
                            addr_space="Shared")
    ar_in = nc.dram_tensor("ar_in", [NGRAPH, OUT], F32)
    ar_out = nc.dram_tensor("ar_out", [NGRAPH, OUT], F32, addr_space="Shared")

    rg = [list(range(NCORES))]

    with tile.TileContext(nc) as tc:
        with (
            tc.tile_pool(name="const", bufs=1) as cpool,
            tc.tile_pool(name="unm", bufs=1) as upool,
            tc.tile_pool(name="uT", bufs=3) as utpool,
            tc.tile_pool(name="GA", bufs=2) as gpoolA,
            tc.tile_pool(name="GB", bufs=2) as gpoolB,
            tc.tile_pool(name="small", bufs=2) as spool,
            tc.tile_pool(name="ps_tr", bufs=2, space="PSUM") as ps_tr,
            tc.tile_pool(name="ps_mm", bufs=2, space="PSUM") as ps_mm,
            tc.tile_pool(name="ps_s", bufs=2, space="PSUM") as ps_s,
            tc.tile_pool(name="ps_end", bufs=1, space="PSUM") as ps_end,
        ):
            # ---- constants
            wt = cpool.tile([128, 3 * H], F32)
            nc.sync.dma_start(wt[:], wt_d[:])
            wh = cpool.tile([128, OUT], F32)
            nc.sync.dma_start(wh[:], wh_d[:])
            bvec = cpool.tile([128, 3], F32)
            nc.sync.dma_start(bvec[:], bvec_d[:])
            bhb = cpool.tile([NGRAPH, OUT], F32)
            nc.sync.dma_start(bhb[:], bhb_d[:])
            giA = cpool.tile([128, NGI * 64], I16)
            nc.sync.dma_start(giA[:], giA_d[:])
            giB = cpool.tile([128, NGI * 64], I16)
            nc.sync.dma_start(giB[:], giB_d[:])
            ucols = cpool.tile([128, ZOFF + 8], F32)
            nc.sync.dma_start(ucols[:], ucols_d[:])
            dinv = cpool.tile([128, TP], F32)
            nc.sync.dma_start(dinv[:], dinv_d[:])
            pmat = cpool.tile([128, TP * NGRAPH], F32)
            nc.sync.dma_start(pmat[:], pmat_d[:])
            ident = cpool.tile([128, 128], F32)
            make_identity(nc, ident[:])
            hT = cpool.tile([128, P_pos], F32)

            # ---- load x (pre-permuted node-major) and transpose to hT
            xg = upool.tile([128, TP * 128], F32, tag="unm")
            nc.sync.dma_start(
                xg[:].rearrange("p (t f) -> p t f", f=128),
                xg_d.ap().rearrange("(t p) f -> p t f", p=128))
            for t in range(TP):
                trp = ps_tr.tile([128, 128], F32, tag="tr")
                nc.tensor.transpose(trp[:], xg[:, t * 128:(t + 1) * 128], ident[:])
                nc.vector.tensor_copy(hT[:, t * 128:(t + 1) * 128], trp[:])

            # ---- layers
            qctr = 0
            for l in range(3):
                u_nm = upool.tile([128, TP * 128], F32, tag="unm")
                for g in range(NST):
                    psu = ps_mm.tile([128, PSUM_COLS], F32, tag="mm")
                    nc.tensor.matmul(
                        psu[:], lhsT=wt[:, l * H:(l + 1) * H],
                        rhs=hT[:, g * PSUM_COLS:(g + 1) * PSUM_COLS],
                        start=True, stop=True)
                    uT = utpool.tile([128, PSUM_COLS], F32, tag="uT")
                    nc.vector.tensor_copy(uT[:], psu[:])
                    for tt in range(PSUM_COLS // 128):
                        t = g * (PSUM_COLS // 128) + tt
                        trp = ps_tr.tile([128, 128], F32, tag="tr")
                        nc.tensor.transpose(
                            trp[:], uT[:, tt * 128:(tt + 1) * 128], ident[:])
                        nc.vector.tensor_scalar_mul(
                            u_nm[:, t * 128:(t + 1) * 128], trp[:],
                            dinv[:, t:t + 1])
                nc.sync.dma_start(
                    out=u_shard.ap().rearrange("(t p) f -> p t f", p=128),
                    in_=u_nm[:].rearrange("p (t f) -> p t f", f=128))
                nc.gpsimd.collective_compute(
                    "AllGather", mybir.AluOpType.bypass, replica_groups=rg,
                    ins=[u_shard.ap().opt()], outs=[u_full.ap().opt()])

                sps = None
                GA = GB = None
                nlast = 0
                for j in range(NBINS):
                    if j % GB_BINS == 0:
                        b = j // GB_BINS
                        n = min(GB_BINS, NBINS - j)
                        nlast = n
                        GA = gpoolA.tile([128, GB_BINS * 128], F32, tag="GA")
                        nc.gpsimd.dma_gather(
                            GA[:, :n * 128].rearrange("p (c f) -> p c f", f=128),
                            u_full[:HALF_ROW, :],
                            giA[:, b * 64:b * 64 + n * 8],
                            n * 128, n * 128, 128, queue_num=qctr % NQ)
                        qctr += 1
                        GB = gpoolB.tile([128, GB_BINS * 128], F32, tag="GB")
                        nc.gpsimd.dma_gather(
                            GB[:, :n * 128].rearrange("p (c f) -> p c f", f=128),
                            u_full[HALF_ROW:, :],
                            giB[:, b * 64:b * 64 + n * 8],
                            n * 128, n * 128, 128, queue_num=qctr % NQ)
                        qctr += 1
                    st, k = divmod(j, ST_BINS)
                    if k == 0:
                        sps = ps_s.tile([128, PSUM_COLS], F32, tag="s")
                    jl = j % GB_BINS
                    colA = k * M_COLS
                    nc.tensor.matmul(
                        sps[:, colA:colA + M_COLS],
                        lhsT=GA[:, jl * 128:(jl + 1) * 128],
                        rhs=ucols[:, j * M_COLS:(j + 1) * M_COLS],
                        start=True, stop=False)
                    nc.tensor.matmul(
                        sps[:, colA:colA + M_COLS],
                        lhsT=GB[:, jl * 128:(jl + 1) * 128],
                        rhs=ucols[:, UC_B + j * M_COLS:UC_B + (j + 1) * M_COLS],
                        start=False, stop=True)
                    if k == ST_BINS - 1:
                        nc.tensor.matmul(
                            sps[:, ST_BINS * M_COLS:PSUM_COLS],
                            lhsT=GB[:, jl * 128:(jl + 1) * 128],
                            rhs=ucols[:, ZOFF:ZOFF + 8],
                            start=True, stop=True)
                        dst_sl = hT[:, st * PSUM_COLS:(st + 1) * PSUM_COLS]
                        if l < 2:
                            nc.scalar.activation(
                                dst_sl, sps[:],
                                mybir.ActivationFunctionType.Relu,
                                bias=bvec[:, l:l + 1])
                        else:
                            nc.vector.tensor_scalar_add(
                                dst_sl, sps[:], bvec[:, l:l + 1])

            # ---- global mean pool (count folded into pmat) + head
            plp = ps_end.tile([NGRAPH, 128], F32, tag="pool")
            for t in range(TP):
                trp = ps_tr.tile([128, 128], F32, tag="tr")
                nc.tensor.transpose(trp[:], hT[:, t * 128:(t + 1) * 128], ident[:])
                h_nm = spool.tile([128, 128], F32, tag="hnm")
                nc.vector.tensor_copy(h_nm[:], trp[:])
                nc.tensor.matmul(
                    plp[:], lhsT=pmat[:, t * NGRAPH:(t + 1) * NGRAPH],
                    rhs=h_nm[:], start=(t == 0), stop=(t == TP - 1))
            pool_nm = spool.tile([NGRAPH, 128], F32, tag="plnm")
            nc.vector.tensor_copy(pool_nm[:], plp[:])
            trp2 = ps_tr.tile([128, NGRAPH], F32, tag="tr")
            nc.tensor.transpose(trp2[:], pool_nm[:], ident[:NGRAPH, :NGRAPH])
            poolT = spool.tile([128, NGRAPH], F32, tag="plT")
            nc.vector.tensor_copy(poolT[:], trp2[:])
            hdp = ps_end.tile([NGRAPH, OUT], F32, tag="head")
            nc.tensor.matmul(hdp[:], lhsT=poolT[:], rhs=wh[:], start=True,
                             stop=True)
            hd = spool.tile([NGRAPH, OUT], F32, tag="hd")
            nc.vector.tensor_copy(hd[:], hdp[:])
            nc.sync.dma_start(ar_in[:], hd[:])
            nc.gpsimd.collective_compute(
                "AllReduce", mybir.AluOpType.add, replica_groups=rg,
                ins=[ar_in.ap().opt()], outs=[ar_out.ap().opt()])
            res = spool.tile([NGRAPH, OUT], F32, tag="res")
            nc.sync.dma_start(res[:], ar_out[:])
            nc.vector.tensor_add(res[:], res[:], bhb[:])
            nc.sync.dma_start(out_d[:], res[:])

    nc.compile()
    return nc


_CACHE = {}


def _get_compiled(meta_key, meta):
    if meta_key not in _CACHE:
        _CACHE[meta_key] = _build(meta)
    return _CACHE[meta_key]


def kernel(x, edge_index, batch, W0, b0, W1, b1, W2, b2, Wh, bh, **_ignored):
    x = np.ascontiguousarray(np.asarray(x, np.float32))
    meta, per_core = _preprocess(edge_index, batch)
    nc = _get_compiled((meta["NBINS"], meta["P_pos"]), meta)

    wt = np.concatenate([np.asarray(W0, np.float32),
                         np.asarray(W1, np.float32),
                         np.asarray(W2, np.float32)], axis=1)
    bvec = np.stack([np.asarray(b0, np.float32), np.asarray(b1, np.float32),
                     np.asarray(b2, np.float32)], axis=1)
    wh = np.asarray(Wh, np.float32)
    bhb = np.tile(np.asarray(bh, np.float32)[None, :], (NGRAPH, 1))

    in_maps = []
    for c in range(NCORES):
        pc = per_core[c]
        xg = np.zeros((meta["P_pos"], 128), np.float32)
        xg[pc["has_node"]] = x[pc["node_order"][pc["has_node"]]]
        in_maps.append({
            "xg": xg, "wt": wt, "wh": wh, "bvec": bvec, "bhb": bhb,
            "gidxA": pc["gidxA"], "gidxB": pc["gidxB"], "ucols": pc["ucols"],
            "dinv": pc["dinv_col"], "pmat": pc["pmat"],
        })

    res = run_bass_kernel_spmd(nc, in_maps, core_ids=list(range(NCORES)))
    return np.asarray(res.results[0]["out"], np.float32)

